# revision 1
# baseline (speedup 1.0000x reference)
"""EvolveGCN (2x GCNConv+GRU + linear head) on 8 Trainium2 NeuronCores.

Strategy: dst-sharded graph parallel. Each core owns 12500 destination
nodes (padded to 12544). Dense per-node compute is feature-major
([128 feat partitions, nodes free]). The GCN propagate is done as
PE matmuls: per 128-edge window w, psum[128f, W] += M_w.T @ S_w, where
M_w is a dma_gather'ed message tile (rows = dinv[src]*t[src]) and S_w is
a host-built selection matrix carrying dinv[dst] weights. Tables are
exchanged between layers with one AllGather.
"""
import sys
import types

import numpy as np

sys.path.insert(0, "/opt/trn_rl_repo")

N = 100000
E = 1600000
IN = 165
H = 128
NC = 8
SH = 12500
SHP = 12544           # 98 * 128
NTAB = NC * SHP       # 100352
GRP = 512
NGRP = 25             # 24x512 + 1x256
GW = [512] * 24 + [256]
WIN = 25088           # int16 gather window (4 windows cover NTAB)
NWIN = 4
CHKD = False


def _install_ntff_hook():
    if "antenv.axon_hooks" in sys.modules:
        return
    try:
        import antenv
        mod = types.ModuleType("antenv.axon_hooks")
        mod._hook = None
        mod.set_axon_ntff_profile_hook = lambda h: setattr(mod, "_hook", h)
        mod.get_axon_ntff_profile_hook = lambda: mod._hook
        sys.modules["antenv.axon_hooks"] = mod
        antenv.axon_hooks = mod
        from trn_agent_boot.trn_boot import _ntff_profile_via_ctypes
        mod.set_axon_ntff_profile_hook(
            _ntff_profile_via_ctypes("/opt/axon/libaxon_pjrt.so"))
    except Exception:
        pass


def _split_excess_waits(nc, bass, max_waits=1, kinds=("InstDrain",)):
    """This walrus build rejects InstDrain with >1 sem waits; hoist extras
    onto standalone event-semaphore instructions placed just before."""
    wait_op_map = {"sem-ge-imm": "sem-ge", "sem-eq-imm": "sem-eq"}
    for bb in nc.main_func.blocks:
        insts = bb.instructions
        i = 0
        while i < len(insts):
            ins = insts[i]
            if (type(ins).__name__ in kinds and ins.sync_info is not None
                    and len(ins.sync_info.on_wait) > max_waits):
                waits = list(ins.sync_info.on_wait)
                ins.sync_info.on_wait = waits[:max_waits]
                eng = nc.engines[ins.engine]
                new_insts = []
                for w in waits[max_waits:]:
                    sem = bass.SemaphoreHandle(w.ant_name or "s", w.id)
                    bi = eng.wait_op(sem, w.wait_value, wait_op_map[w.wait_mode])
                    popped = None
                    for b2 in nc.main_func.blocks:
                        if b2.instructions and b2.instructions[-1] is bi.ins:
                            popped = b2.instructions.pop()
                            break
                    assert popped is bi.ins
                    new_insts.append(popped)
                for k, ni in enumerate(new_insts):
                    insts.insert(i + k, ni)
                i += len(new_insts)
            i += 1


def _newid(v):
    return (v // SH) * SHP + (v % SH)


def _preprocess(edge_index):
    """Build per-core gather index streams + selection matrix streams."""
    e0 = np.asarray(edge_index[0], dtype=np.int64)
    e1 = np.asarray(edge_index[1], dtype=np.int64)
    deg = np.bincount(e1, minlength=N).astype(np.float64) + 1.0
    dinv = (1.0 / np.sqrt(deg)).astype(np.float32)

    src = np.concatenate([e0, np.arange(N, dtype=np.int64)])
    dst = np.concatenate([e1, np.arange(N, dtype=np.int64)])
    w = dinv[dst]

    src_n = _newid(src)
    core = dst // SH
    dstloc = dst % SH

    per_core = []
    for c in range(NC):
        m = core == c
        s, dl, ww = src_n[m], dstloc[m], w[m].astype(np.float32)
        g = dl // GRP
        ch = s // WIN
        order = np.lexsort((s, dl, ch, g))
        s, dl, ww, g, ch = s[order], dl[order], ww[order], g[order], ch[order]

        idx_cols, s_cols, calls = [], [], []
        key = g * NWIN + ch
        bounds = np.flatnonzero(np.diff(key)) + 1
        starts = np.concatenate([[0], bounds])
        ends = np.concatenate([bounds, [len(s)]])
        idx_off = 0
        s_off = 0
        for a, b in zip(starts, ends):
            gg, cc = int(g[a]), int(ch[a])
            n = b - a
            npad = -n % 128
            ntot = n + npad
            ss = np.concatenate([s[a:b] - cc * WIN, np.zeros(npad, np.int64)])
            dd = np.concatenate([dl[a:b], np.full(npad, dl[b - 1], np.int64)])
            vv = np.concatenate([ww[a:b], np.zeros(npad, np.float32)])
            idx_cols.append(ss.astype(np.int16))
            windows = []
            for k in range(ntot // 128):
                dk = dd[k * 128:(k + 1) * 128]
                vk = vv[k * 128:(k + 1) * 128]
                lo = int(dk.min())
                W = int(dk.max()) - lo + 1
                sb = np.zeros((128, W), np.float32)
                sb[np.arange(128), dk - lo] = vk
                s_cols.append(sb)
                windows.append((s_off, W, lo - gg * GRP))
                s_off += W
            calls.append((gg, cc, idx_off, ntot, windows))
            idx_off += ntot

        idx_flat = np.concatenate(idx_cols)
        idx_wrapped = np.tile(idx_flat.reshape(-1, 16).T, (NC, 1)).copy()
        s_flat = np.ascontiguousarray(np.concatenate(s_cols, axis=1))
        per_core.append(dict(idx=idx_wrapped, s=s_flat, calls=calls,
                             n_idx=idx_off, n_scols=s_off))
    return dinv, per_core


def _build_program(bass, bacc, mybir, tile, meta):
    """Build the SPMD bass program. meta = per-core call/window structure
    (identical instruction COUNT per core is required for SPMD; counts may
    differ -> we use core 0's structure? No: all cores share one program.
    We therefore pad every core's structure to the max over cores."""
    nc = bacc.Bacc("TRN2", target_bir_lowering=False, debug=False)
    dt = mybir.dt
    f32 = dt.float32

    def din(name, shape, dtype=f32):
        return nc.dram_tensor(name, shape, dtype, kind="ExternalInput").ap()

    n_idx, n_scols, calls = meta["n_idx"], meta["n_scols"], meta["calls"]
    nwmax = meta["nwmax"]

    xT_hi = din("xT_hi", [128, SHP])
    xT_lo = din("xT_lo", [IN - 128, SHP])
    W0T_hi = din("W0T_hi", [128, H])
    W0T_lo = din("W0T_lo", [IN - 128, H])
    W1T = din("W1T", [H, H])
    WihT = [din(f"WihT{li}", [H, 3 * H]) for li in range(2)]
    WlinT = din("WlinT", [H, 2])
    bcol = [din(f"bcol{li}", [128, 1]) for li in range(2)]
    brc = [din(f"brc{li}", [128, 1]) for li in range(2)]
    bzc = [din(f"bzc{li}", [128, 1]) for li in range(2)]
    bnc = [din(f"bnc{li}", [128, 1]) for li in range(2)]
    bhnc = [din(f"bhnc{li}", [128, 1]) for li in range(2)]
    dinv_col = din("dinv_col", [128, SHP // 128])
    blin_t = din("blin_t", [128, 2])
    idx_d = din("idx_d", [128, n_idx // 16], dt.int16)
    s_d = din("s_d", [128, n_scols])

    out = nc.dram_tensor("out", [128, (SHP // 128) * 2], f32,
                         kind="ExternalOutput").ap()
    import os as _os
    _dbg = bool(int(_os.environ.get("KERNEL_DEBUG", "0")))
    if _dbg:
        dbg_t0 = nc.dram_tensor("dbg_t0", [SHP, H], f32, kind="ExternalOutput").ap()
        dbg_tab = nc.dram_tensor("dbg_tab", [2048, H], f32, kind="ExternalOutput").ap()
        dbg_h = nc.dram_tensor("dbg_h", [128, SHP], f32, kind="ExternalOutput").ap()
        dbg_t1 = nc.dram_tensor("dbg_t1", [SHP, H], f32, kind="ExternalOutput").ap()
        dbg_tab1 = nc.dram_tensor("dbg_tab1", [2048, H], f32, kind="ExternalOutput").ap()
        dbg_g1 = nc.dram_tensor("dbg_g1", [128, SHP], f32, kind="ExternalOutput").ap()

    tsh = [nc.dram_tensor(f"tshard{li}", [SHP, H], f32) for li in range(2)]
    Ttab = [nc.dram_tensor(f"Ttab{li}", [NTAB, H], f32, addr_space="Shared")
            for li in range(2)]

    from concourse.masks import make_identity

    with tile.TileContext(nc) as tc:
        with (
            tc.tile_pool(name="const", bufs=1) as cp,
            tc.tile_pool(name="sb", bufs=3) as sp,
            tc.tile_pool(name="big", bufs=1) as bigp,
            tc.tile_pool(name="gat", bufs=3) as gatp,
            tc.tile_pool(name="ps", bufs=2, space="PSUM") as pp,
            tc.tile_pool(name="ps2", bufs=2, space="PSUM") as pp2,
            tc.tile_pool(name="ps3", bufs=2, space="PSUM") as pp3,
            tc.tile_pool(name="dram", bufs=1, space="DRAM") as _dp,
        ):
            ident = cp.tile([128, 128], f32)
            make_identity(nc, ident[:])
            # resident constants
            w0hi = cp.tile([128, H], f32)
            nc.sync.dma_start(out=w0hi[:], in_=W0T_hi[:])
            w0lo = cp.tile([IN - 128, H], f32)
            nc.sync.dma_start(out=w0lo[:], in_=W0T_lo[:])
            w1 = cp.tile([H, H], f32)
            nc.sync.dma_start(out=w1[:], in_=W1T[:])
            wih = [cp.tile([H, 3 * H], f32, tag=f"wih{li}", name=f"wih{li}") for li in range(2)]
            for li in range(2):
                nc.sync.dma_start(out=wih[li][:], in_=WihT[li][:])
            wlin = cp.tile([H, 2], f32)
            nc.sync.dma_start(out=wlin[:], in_=WlinT[:])
            bc = [cp.tile([128, 1], f32, name=f"bc{li}") for li in range(2)]
            br = [cp.tile([128, 1], f32, name=f"br{li}") for li in range(2)]
            bz = [cp.tile([128, 1], f32, name=f"bz{li}") for li in range(2)]
            bn = [cp.tile([128, 1], f32, name=f"bn{li}") for li in range(2)]
            bhn = [cp.tile([128, 1], f32, name=f"bhn{li}") for li in range(2)]
            for li in range(2):
                nc.sync.dma_start(out=bc[li][:], in_=bcol[li][:])
                nc.sync.dma_start(out=br[li][:], in_=brc[li][:])
                nc.sync.dma_start(out=bz[li][:], in_=bzc[li][:])
                nc.sync.dma_start(out=bn[li][:], in_=bnc[li][:])
                nc.sync.dma_start(out=bhn[li][:], in_=bhnc[li][:])
            dvc = cp.tile([128, SHP // 128], f32)
            nc.sync.dma_start(out=dvc[:], in_=dinv_col[:])
            blt = cp.tile([128, 2], f32)
            nc.sync.dma_start(out=blt[:], in_=blin_t[:])

            hT = bigp.tile([128, SHP], f32, tag="hT")      # h1.T resident
            y_sb = bigp.tile([128, (SHP // 128) * 2], f32, tag="ysb")

            ACT = mybir.ActivationFunctionType

            def dense_to_table(li, rhs_tiles_fn):
                """t.T = W @ h.T per group; transpose; scale dinv; DMA shard."""
                for g in range(NGRP):
                    gw = GW[g]
                    g0 = g * GRP
                    pt = pp.tile([128, GRP], f32, tag="pt")
                    if li == 0:
                        xh = sp.tile([128, GRP], f32, tag="xh")
                        nc.sync.dma_start(out=xh[:, :gw],
                                          in_=xT_hi[:, g0:g0 + gw])
                        xl = sp.tile([IN - 128, GRP], f32, tag="xl")
                        nc.sync.dma_start(out=xl[:, :gw],
                                          in_=xT_lo[:, g0:g0 + gw])
                        nc.tensor.matmul(pt[:, :gw], w0hi[:], xh[:, :gw],
                                         start=True, stop=False)
                        nc.tensor.matmul(pt[:, :gw], w0lo[:], xl[:, :gw],
                                         start=False, stop=True)
                    else:
                        nc.tensor.matmul(pt[:, :gw], w1[:],
                                         hT[:, g0:g0 + gw],
                                         start=True, stop=True)
                    tsb = sp.tile([128, GRP], f32, tag="tsb")
                    nc.scalar.copy(tsb[:, :gw], pt[:, :gw])
                    for t in range(gw // 128):
                        ptr = pp3.tile([128, 128], f32, tag="ptr")
                        nc.tensor.transpose(
                            out=ptr[:], in_=tsb[:, 128 * t:128 * (t + 1)],
                            identity=ident[:])
                        trs = sp.tile([128, 128], f32, tag="trs")
                        col = g * (GRP // 128) + t
                        nc.vector.tensor_scalar_mul(
                            trs[:], ptr[:], dvc[:, col:col + 1])
                        nc.sync.dma_start(
                            out=tsh[li][g0 + 128 * t:g0 + 128 * (t + 1), :],
                            in_=trs[:])

            def dbg_g1_slice(g0, gw):
                return dbg_g1[:, g0:g0 + gw]

            def edge_phase(li):
                """agg per group via gather + S matmuls; GRU epilogue."""
                # group calls by g
                for g in range(NGRP):
                    gw = GW[g]
                    gcalls = [cl for cl in calls if cl[0] == g]
                    pagg = pp.tile([128, GRP], f32, tag="pagg")
                    nc.vector.memset(pagg[:, :gw], 0.0)
                    for (gg, cc, ioff, ntot, windows) in gcalls:
                        soff0 = windows[0][0]
                        scw = windows[-1][0] + windows[-1][1] - soff0
                        assert scw <= 2560, scw
                        ssl = sp.tile([128, 2560], f32, tag="ssl", bufs=3,
                                      name=f"ssl_{li}_{gg}_{cc}_{ioff}")
                        nc.sync.dma_start(out=ssl[:, :scw],
                                          in_=s_d[:, soff0:soff0 + scw])
                        nw = ntot // 128
                        assert nw <= nwmax
                        it = sp.tile([128, nwmax * 8], dt.int16, tag="it")
                        nc.sync.dma_start(
                            out=it[:, :ntot // 16],
                            in_=idx_d[:, ioff // 16:(ioff + ntot) // 16])
                        mt = gatp.tile([128, nwmax * H], f32, tag="mt")
                        nc.gpsimd.dma_gather(
                            out_ap=mt[:, :nw * H].rearrange(
                                "p (c d) -> p c d", d=H),
                            in_ap=Ttab[li][cc * WIN:(cc + 1) * WIN, :],
                            idxs_ap=it[:, :ntot // 16],
                            num_idxs=ntot, num_idxs_reg=ntot,
                            elem_size=H,
                            single_packet=bool(ntot <= 1024))
                        for k, (soff, W, o) in enumerate(windows):
                            nc.tensor.matmul(
                                pagg[:, o:o + W],
                                mt[:, k * H:(k + 1) * H],
                                ssl[:, soff - soff0:soff - soff0 + W],
                                start=False, stop=False)
                    g0 = g * GRP
                    hg = sp.tile([128, GRP], f32, tag="hg")
                    nc.scalar.activation(hg[:, :gw], pagg[:, :gw], ACT.Relu,
                                         bias=bc[li][:], scale=1.0)
                    if _dbg and li == 1:
                        nc.sync.dma_start(out=dbg_g1_slice(g0, gw), in_=hg[:, :gw])
                    # GRU gates
                    pgr = pp2.tile([128, GRP], f32, tag="pg")
                    nc.tensor.matmul(pgr[:, :gw], wih[li][:, 0:H],
                                     hg[:, :gw], start=True, stop=True)
                    rt = sp.tile([128, GRP], f32, tag="rt")
                    nc.scalar.activation(rt[:, :gw], pgr[:, :gw], ACT.Sigmoid,
                                         bias=br[li][:], scale=1.0)
                    pgz = pp2.tile([128, GRP], f32, tag="pg")
                    nc.tensor.matmul(pgz[:, :gw], wih[li][:, H:2 * H],
                                     hg[:, :gw], start=True, stop=True)
                    zt = sp.tile([128, GRP], f32, tag="zt")
                    nc.scalar.activation(zt[:, :gw], pgz[:, :gw], ACT.Sigmoid,
                                         bias=bz[li][:], scale=-1.0)
                    pgn = pp2.tile([128, GRP], f32, tag="pg")
                    nc.tensor.matmul(pgn[:, :gw], wih[li][:, 2 * H:3 * H],
                                     hg[:, :gw], start=True, stop=True)
                    tmp = sp.tile([128, GRP], f32, tag="tmp")
                    nc.vector.tensor_scalar_mul(tmp[:, :gw], rt[:, :gw],
                                                bhn[li][:])
                    st = sp.tile([128, GRP], f32, tag="st")
                    nc.vector.tensor_add(st[:, :gw], pgn[:, :gw], tmp[:, :gw])
                    nt = sp.tile([128, GRP], f32, tag="nt")
                    nc.scalar.activation(nt[:, :gw], st[:, :gw], ACT.Tanh,
                                         bias=bn[li][:], scale=1.0)
                    if li == 0:
                        nc.vector.tensor_mul(hT[:, g0:g0 + gw], zt[:, :gw],
                                             nt[:, :gw])
                    else:
                        h2 = sp.tile([128, GRP], f32, tag="h2")
                        nc.vector.tensor_mul(h2[:, :gw], zt[:, :gw],
                                             nt[:, :gw])
                        for t in range(gw // 128):
                            py = pp3.tile([128, 128], f32, tag="ptr")
                            nc.tensor.matmul(py[:, :2],
                                             h2[:, 128 * t:128 * (t + 1)],
                                             wlin[:], start=True, stop=True)
                            col = g * (GRP // 128) + t
                            nc.vector.tensor_add(
                                y_sb[:, 2 * col:2 * col + 2],
                                py[:, :2], blt[:])

            # ---- layer 0 ----
            dense_to_table(0, None)
            nc.gpsimd.collective_compute(
                "AllGather", mybir.AluOpType.bypass,
                replica_groups=[list(range(NC))],
                ins=[tsh[0][:]], outs=[Ttab[0][:]])
            if _dbg:
                nc.gpsimd.dma_start(out=dbg_t0[:], in_=tsh[0][:])
                nc.gpsimd.dma_start(out=dbg_tab[:], in_=Ttab[0][40960:43008, :])
            edge_phase(0)
            if _dbg:
                nc.sync.dma_start(out=dbg_h[:], in_=hT[:])
            # ---- layer 1 ----
            dense_to_table(1, None)
            nc.gpsimd.collective_compute(
                "AllGather", mybir.AluOpType.bypass,
                replica_groups=[list(range(NC))],
                ins=[tsh[1][:]], outs=[Ttab[1][:]])
            if _dbg:
                nc.gpsimd.dma_start(out=dbg_t1[:], in_=tsh[1][:])
                nc.gpsimd.dma_start(out=dbg_tab1[:], in_=Ttab[1][40960:43008, :])
            edge_phase(1)
            nc.sync.dma_start(out=out[:], in_=y_sb[:])

    _split_excess_waits(nc, bass)
    nc.finalize()
    return nc


def kernel(**inputs):
    _install_ntff_hook()
    import concourse.bass as bass
    import concourse.bacc as bacc
    import concourse.mybir as mybir
    import concourse.tile as tile
    from concourse.bass_utils import run_bass_kernel_spmd

    x = np.asarray(inputs["x"], np.float32)
    edge_index = np.asarray(inputs["edge_index"])
    dinv, per_core = _preprocess(edge_index)

    # pad all cores to the same structure: use each core's own metadata but
    # the program is SPMD (one program). We must make the structure
    # identical: pad n_idx / windows to the max. Simplest: build with
    # per-core maximal structure by unioning call shapes.
    # All cores have the same (g, ch) call set (all 100 combos present with
    # high probability); pad each call's ntot and window list to the max
    # over cores, and pad S columns/windows to match.
    key_set = sorted({(cl[0], cl[1]) for pc in per_core for cl in pc["calls"]})
    call_shape = {}
    for kk in key_set:
        mx_nt = 0
        for pc in per_core:
            for cl in pc["calls"]:
                if (cl[0], cl[1]) == kk:
                    mx_nt = max(mx_nt, cl[3])
        call_shape[kk] = mx_nt
    # uniform window column bounds across cores: o_u = min o, end = max o+W
    win_b = {}
    for kk in key_set:
        nww = call_shape[kk] // 128
        for wi in range(nww):
            lo, hi = None, None
            for pc in per_core:
                for cl in pc["calls"]:
                    if (cl[0], cl[1]) == kk and wi < len(cl[4]):
                        _, W_, o_ = cl[4][wi]
                        lo = o_ if lo is None else min(lo, o_)
                        hi = o_ + W_ if hi is None else max(hi, o_ + W_)
            if lo is None:
                lo, hi = 0, 1
            win_b[(kk, wi)] = (lo, hi - lo)

    # rebuild uniform per-core streams
    uni = None
    idx_arrs, s_arrs = [], []
    for pc in per_core:
        bycall = {(cl[0], cl[1]): cl for cl in pc["calls"]}
        idx_cols, s_cols, calls = [], [], []
        ioff = 0
        soff = 0
        for kk in key_set:
            gg, cc = kk
            ntot = call_shape[kk]
            nww = ntot // 128
            windows = []
            if kk in bycall:
                _, _, io0, nt0, ws0 = bycall[kk]
                iflat = pc["idx"][:16].T.reshape(-1)[io0:io0 + nt0]
            else:
                nt0, ws0 = 0, []
                iflat = np.zeros(0, np.int16)
            iful = np.concatenate(
                [iflat, np.zeros(ntot - nt0, np.int16)])
            idx_cols.append(iful)
            for wi in range(nww):
                o_u, Wp = win_b[(kk, wi)]
                sb = np.zeros((128, Wp), np.float32)
                if wi < len(ws0):
                    s0, W0_, o_ = ws0[wi]
                    sb[:, o_ - o_u:o_ - o_u + W0_] = pc["s"][:, s0:s0 + W0_]
                windows.append((soff, Wp, o_u))
                s_cols.append(sb)
                soff += Wp
            # cap calls at 12 windows to bound the gather tile
            for sc0 in range(0, nww, 21):
                wsub = windows[sc0:sc0 + 21]
                nsub = 128 * len(wsub)
                calls.append((gg, cc, ioff, nsub, wsub))
                ioff += nsub
        idx_flat = np.concatenate(idx_cols)
        idx_arrs.append(np.tile(idx_flat.reshape(-1, 16).T, (NC, 1)).copy())
        s_arrs.append(np.ascontiguousarray(np.concatenate(s_cols, axis=1)))
        if uni is None:
            nwmax = max(cl[3] // 128 for cl in calls)
            uni = dict(calls=calls, n_idx=ioff, n_scols=soff, nwmax=nwmax)

    nc = _build_program(bass, bacc, mybir, tile, uni)

    # stage per-core inputs
    W0 = np.asarray(inputs["W0"], np.float32)
    W1 = np.asarray(inputs["W1"], np.float32)
    Wlin = np.asarray(inputs["Wlin"], np.float32)
    in_maps = []
    for c in range(NC):
        ids = np.arange(c * SH, (c + 1) * SH)
        xs = np.zeros((SHP, IN), np.float32)
        xs[:SH] = x[ids]
        xT = np.ascontiguousarray(xs.T)
        bias_stage = {}
        for li in range(2):
            bih = np.asarray(inputs[f"bih{li}"], np.float32)
            bhh = np.asarray(inputs[f"bhh{li}"], np.float32)
            bias_stage[f"bcol{li}"] = np.asarray(inputs[f"b{li}"], np.float32).reshape(128, 1)
            bias_stage[f"brc{li}"] = (bih[:H] + bhh[:H]).reshape(128, 1)
            bias_stage[f"bzc{li}"] = (-(bih[H:2 * H] + bhh[H:2 * H])).reshape(128, 1)
            bias_stage[f"bnc{li}"] = bih[2 * H:].reshape(128, 1)
            bias_stage[f"bhnc{li}"] = bhh[2 * H:].reshape(128, 1)
        dv = np.zeros(SHP, np.float32)
        dv[:SH] = dinv[ids]
        in_maps.append({
            "xT_hi": xT[:128], "xT_lo": xT[128:],
            "W0T_hi": np.ascontiguousarray(W0.T[:128]),
            "W0T_lo": np.ascontiguousarray(W0.T[128:]),
            "W1T": np.ascontiguousarray(W1.T),
            "WihT0": np.ascontiguousarray(np.asarray(inputs["Wih0"], np.float32).T),
            "WihT1": np.ascontiguousarray(np.asarray(inputs["Wih1"], np.float32).T),
            "WlinT": np.ascontiguousarray(Wlin.T),
            **bias_stage,
            "dinv_col": np.ascontiguousarray(dv.reshape(SHP // 128, 128).T),
            "blin_t": np.tile(np.asarray(inputs["blin"], np.float32), (128, 1)),
            "idx_d": idx_arrs[c],
            "s_d": s_arrs[c],
        })

    res = run_bass_kernel_spmd(nc, in_maps, list(range(NC)),
                               trace=bool(int(__import__("os").environ.get(
                                   "KERNEL_TRACE", "0"))))
    kernel.last_results = res
    y = np.zeros((N, 2), np.float32)
    for c in range(NC):
        o = res.results[c]["out"]  # [128, 98*2]
        yy = o.reshape(128, SHP // 128, 2).transpose(1, 0, 2).reshape(SHP, 2)
        y[c * SH:(c + 1) * SH] = yy[:SH]
    return y



# revision 23
# speedup vs baseline: 2.2679x; 2.2679x over previous
"""EvolveGCN (2x GCNConv+GRU + linear head) on 8 Trainium2 NeuronCores.

Strategy: dst-sharded graph parallel. Each core owns 12500 destination
nodes (padded to 12544). Dense per-node compute is feature-major
([128 feat partitions, nodes free]) in fp32. The GCN propagate uses a
table of per-node rows dinv[src]*t[src] stored as compensated fp16
hi+lo pairs ([NTAB, 2H] fp16, hi|lo packed per row, 512B/row). Edge
messages are fetched with dma_gather (4-way parallel across the 4 SWDGE
queues / Q7 core pairs), multiplied against host-built one-hot fp16
selection matrices on the PE (2 matmuls per 128-edge window: hi + lo),
accumulated in fp32 PSUM, then column-scaled by dinv[dst] in fp32.
Gather index streams are padded with -1 (the Q7 ucode trims trailing
negatives, so emission cost tracks the true per-core edge count).
Tables are exchanged between layers with one AllGather per layer.
"""
import sys
import types

import numpy as np

sys.path.insert(0, "/opt/trn_rl_repo")

N = 100000
E = 1600000
IN = 165
H = 128
NC = 8
SH = 12500
SHP = 12544           # 98 * 128
NTAB = NC * SHP       # 100352
GRP = 512
NGRP = 25             # 24x512 + 1x256
GW = [512] * 24 + [256]
WIN = 25088           # int16 gather window (4 windows cover NTAB)
NWIN = 4
NWCAP = 21            # max 128-edge windows per gather call
NQ = int(__import__("os").environ.get("KERNEL_NQ", "4"))
GBUFS = 5             # gather tile pool depth (4 queues in flight + 1)


def _install_ntff_hook():
    if "antenv.axon_hooks" in sys.modules:
        return
    try:
        import antenv
        mod = types.ModuleType("antenv.axon_hooks")
        mod._hook = None
        mod.set_axon_ntff_profile_hook = lambda h: setattr(mod, "_hook", h)
        mod.get_axon_ntff_profile_hook = lambda: mod._hook
        sys.modules["antenv.axon_hooks"] = mod
        antenv.axon_hooks = mod
        from trn_agent_boot.trn_boot import _ntff_profile_via_ctypes
        mod.set_axon_ntff_profile_hook(
            _ntff_profile_via_ctypes("/opt/axon/libaxon_pjrt.so"))
    except Exception:
        pass


def _split_excess_waits(nc, bass, max_waits=1, kinds=("InstDrain",)):
    """This walrus build rejects InstDrain with >1 sem waits; hoist extras
    onto standalone event-semaphore instructions placed just before."""
    wait_op_map = {"sem-ge-imm": "sem-ge", "sem-eq-imm": "sem-eq"}
    for bb in nc.main_func.blocks:
        insts = bb.instructions
        i = 0
        while i < len(insts):
            ins = insts[i]
            if (type(ins).__name__ in kinds and ins.sync_info is not None
                    and len(ins.sync_info.on_wait) > max_waits):
                waits = list(ins.sync_info.on_wait)
                ins.sync_info.on_wait = waits[:max_waits]
                eng = nc.engines[ins.engine]
                new_insts = []
                for w in waits[max_waits:]:
                    sem = bass.SemaphoreHandle(w.ant_name or "s", w.id)
                    bi = eng.wait_op(sem, w.wait_value, wait_op_map[w.wait_mode])
                    popped = None
                    for b2 in nc.main_func.blocks:
                        if b2.instructions and b2.instructions[-1] is bi.ins:
                            popped = b2.instructions.pop()
                            break
                    assert popped is bi.ins
                    new_insts.append(popped)
                for k, ni in enumerate(new_insts):
                    insts.insert(i + k, ni)
                i += len(new_insts)
            i += 1


def _newid(v):
    return (v // SH) * SHP + (v % SH)


def _preprocess(edge_index):
    """Build per-core gather index streams + one-hot selection streams."""
    e0 = np.asarray(edge_index[0], dtype=np.int64)
    e1 = np.asarray(edge_index[1], dtype=np.int64)
    deg = np.bincount(e1, minlength=N).astype(np.float64) + 1.0
    dinv = (1.0 / np.sqrt(deg)).astype(np.float32)

    src = np.concatenate([e0, np.arange(N, dtype=np.int64)])
    dst = np.concatenate([e1, np.arange(N, dtype=np.int64)])

    src_n = _newid(src)
    core = dst // SH
    dstloc = dst % SH

    per_core = []
    for c in range(NC):
        m = core == c
        s, dl = src_n[m], dstloc[m]
        g = dl // GRP
        ch = s // WIN
        order = np.lexsort((s, dl, ch, g))
        s, dl, g, ch = s[order], dl[order], g[order], ch[order]

        idx_cols, s_cols, calls = [], [], []
        key = g * NWIN + ch
        bounds = np.flatnonzero(np.diff(key)) + 1
        starts = np.concatenate([[0], bounds])
        ends = np.concatenate([bounds, [len(s)]])
        idx_off = 0
        s_off = 0
        for a, b in zip(starts, ends):
            gg, cc = int(g[a]), int(ch[a])
            n = b - a
            npad = -n % 128
            ntot = n + npad
            ss = np.concatenate(
                [s[a:b] - cc * WIN, np.full(npad, -1, np.int64)])
            dd = np.concatenate([dl[a:b], np.full(npad, dl[b - 1], np.int64)])
            vv = np.concatenate(
                [np.ones(n, np.float32), np.zeros(npad, np.float32)])
            idx_cols.append(ss.astype(np.int16))
            windows = []
            for k in range(ntot // 128):
                dk = dd[k * 128:(k + 1) * 128]
                vk = vv[k * 128:(k + 1) * 128]
                lo = int(dk.min())
                W = int(dk.max()) - lo + 1
                sb = np.zeros((128, W), np.float16)
                sb[np.arange(128), dk - lo] = vk
                s_cols.append(sb)
                windows.append((s_off, W, lo - gg * GRP))
                s_off += W
            calls.append((gg, cc, idx_off, ntot, windows))
            idx_off += ntot

        idx_flat = np.concatenate(idx_cols)
        idx_wrapped = np.tile(idx_flat.reshape(-1, 16).T, (NC, 1)).copy()
        s_flat = np.ascontiguousarray(np.concatenate(s_cols, axis=1))
        per_core.append(dict(idx=idx_wrapped, s=s_flat, calls=calls,
                             n_idx=idx_off, n_scols=s_off))
    return dinv, per_core


def _build_program(bass, bacc, mybir, tile, meta):
    """Build the SPMD bass program (identical structure for all cores)."""
    nc = bacc.Bacc("TRN2", target_bir_lowering=False, debug=False,
                   num_swdge_queues=NQ)
    dt = mybir.dt
    f32 = dt.float32
    f16 = dt.float16

    def din(name, shape, dtype=f32):
        return nc.dram_tensor(name, shape, dtype, kind="ExternalInput").ap()

    n_idx, n_scols, calls = meta["n_idx"], meta["n_scols"], meta["calls"]
    nwmax = meta["nwmax"]
    sslcap = meta["sslcap"]

    xT_hi = din("xT_hi", [128, SHP])
    xT_lo = din("xT_lo", [IN - 128, SHP])
    W0T_hi = din("W0T_hi", [128, H])
    W0T_lo = din("W0T_lo", [IN - 128, H])
    W1T = din("W1T", [H, H])
    WihT = [din(f"WihT{li}", [H, 3 * H]) for li in range(2)]
    WlinT = din("WlinT", [H, 2])
    bcol = [din(f"bcol{li}", [128, 1]) for li in range(2)]
    brc = [din(f"brc{li}", [128, 1]) for li in range(2)]
    bzc = [din(f"bzc{li}", [128, 1]) for li in range(2)]
    bnc = [din(f"bnc{li}", [128, 1]) for li in range(2)]
    bhnc = [din(f"bhnc{li}", [128, 1]) for li in range(2)]
    dinv_col = din("dinv_col", [128, SHP // 128])
    dv_rep = din("dv_rep", [128, SHP])
    blin_t = din("blin_t", [128, 2])
    idx_d = din("idx_d", [128, n_idx // 16], dt.int16)
    s_d = din("s_d", [128, n_scols], f16)
    ncalls = len(calls)
    cnt_d = din("cnt_d", [1, ncalls], dt.int32)

    out = nc.dram_tensor("out", [128, (SHP // 128) * 2], f32,
                         kind="ExternalOutput").ap()
    _dbg = bool(int(__import__("os").environ.get("KERNEL_DEBUG", "0")))
    if _dbg:
        dbg_hgs = nc.dram_tensor("dbg_hgs", [128, SHP], f32,
                                 kind="ExternalOutput").ap()
        dbg_hg = nc.dram_tensor("dbg_hg", [128, SHP], f32,
                                kind="ExternalOutput").ap()
        dbg_hT = nc.dram_tensor("dbg_hT", [128, SHP], f32,
                                kind="ExternalOutput").ap()

    tsh = [nc.dram_tensor(f"tshard{li}", [SHP, 2 * H], f16) for li in range(2)]
    Ttab = [nc.dram_tensor(f"Ttab{li}", [NTAB, 2 * H], f16,
                           addr_space="Shared") for li in range(2)]

    from concourse.masks import make_identity

    with tile.TileContext(nc) as tc:
        with (
            tc.tile_pool(name="const", bufs=1) as cp,
            tc.tile_pool(name="sb", bufs=3) as sp,
            tc.tile_pool(name="big", bufs=1) as bigp,
            tc.tile_pool(name="gat", bufs=GBUFS) as gatp,
            tc.tile_pool(name="ps", bufs=2, space="PSUM") as pp,
            tc.tile_pool(name="ps2", bufs=2, space="PSUM") as pp2,
            tc.tile_pool(name="ps3", bufs=2, space="PSUM") as pp3,
            tc.tile_pool(name="dram", bufs=1, space="DRAM") as _dp,
        ):
            ident = cp.tile([128, 128], f32)
            make_identity(nc, ident[:])
            # resident constants
            w0hi = cp.tile([128, H], f32)
            nc.sync.dma_start(out=w0hi[:], in_=W0T_hi[:])
            w0lo = cp.tile([IN - 128, H], f32)
            nc.sync.dma_start(out=w0lo[:], in_=W0T_lo[:])
            w1 = cp.tile([H, H], f32)
            nc.sync.dma_start(out=w1[:], in_=W1T[:])
            wih = [cp.tile([H, 3 * H], f32, tag=f"wih{li}", name=f"wih{li}")
                   for li in range(2)]
            for li in range(2):
                nc.sync.dma_start(out=wih[li][:], in_=WihT[li][:])
            wlin = cp.tile([H, 2], f32)
            nc.sync.dma_start(out=wlin[:], in_=WlinT[:])
            bc = [cp.tile([128, 1], f32, name=f"bc{li}") for li in range(2)]
            br = [cp.tile([128, 1], f32, name=f"br{li}") for li in range(2)]
            bz = [cp.tile([128, 1], f32, name=f"bz{li}") for li in range(2)]
            bn = [cp.tile([128, 1], f32, name=f"bn{li}") for li in range(2)]
            bhn = [cp.tile([128, 1], f32, name=f"bhn{li}") for li in range(2)]
            for li in range(2):
                nc.sync.dma_start(out=bc[li][:], in_=bcol[li][:])
                nc.sync.dma_start(out=br[li][:], in_=brc[li][:])
                nc.sync.dma_start(out=bz[li][:], in_=bzc[li][:])
                nc.sync.dma_start(out=bn[li][:], in_=bnc[li][:])
                nc.sync.dma_start(out=bhn[li][:], in_=bhnc[li][:])
            dvc = cp.tile([128, SHP // 128], f32)
            nc.sync.dma_start(out=dvc[:], in_=dinv_col[:])
            blt = cp.tile([128, 2], f32)
            nc.sync.dma_start(out=blt[:], in_=blin_t[:])
            cnt_t = cp.tile([1, ncalls], dt.int32)
            nc.sync.dma_start(out=cnt_t[:], in_=cnt_d[:])

            hT = bigp.tile([128, SHP], f32, tag="hT")      # h1.T resident
            y_sb = bigp.tile([128, (SHP // 128) * 2], f32, tag="ysb")

            ACT = mybir.ActivationFunctionType



            def dense_to_table(li):
                """t.T = W @ h.T per group; transpose; scale dinv;
                split fp16 hi/lo; DMA shard."""
                for g in range(NGRP):
                    gw = GW[g]
                    g0 = g * GRP
                    pt = pp.tile([128, GRP], f32, tag="pt")
                    if li == 0:
                        xh = sp.tile([128, GRP], f32, tag="xh")
                        nc.sync.dma_start(out=xh[:, :gw],
                                          in_=xT_hi[:, g0:g0 + gw])
                        xl = sp.tile([IN - 128, GRP], f32, tag="xl")
                        nc.sync.dma_start(out=xl[:, :gw],
                                          in_=xT_lo[:, g0:g0 + gw])
                        nc.tensor.matmul(pt[:, :gw], w0hi[:], xh[:, :gw],
                                         start=True, stop=False)
                        nc.tensor.matmul(pt[:, :gw], w0lo[:], xl[:, :gw],
                                         start=False, stop=True)
                    else:
                        nc.tensor.matmul(pt[:, :gw], w1[:],
                                         hT[:, g0:g0 + gw],
                                         start=True, stop=True)
                    tsb = sp.tile([128, GRP], f32, tag="tsb")
                    nc.scalar.copy(tsb[:, :gw], pt[:, :gw])
                    for t in range(gw // 128):
                        ptr = pp3.tile([128, 128], f32, tag="ptr")
                        nc.tensor.transpose(
                            out=ptr[:], in_=tsb[:, 128 * t:128 * (t + 1)],
                            identity=ident[:])
                        trs = sp.tile([128, 128], f32, tag="trs")
                        col = g * (GRP // 128) + t
                        nc.vector.tensor_scalar_mul(
                            trs[:], ptr[:], dvc[:, col:col + 1])
                        t2 = sp.tile([128, 2 * H], f16, tag="t2")
                        nc.scalar.copy(t2[:, 0:H], trs[:])
                        nc.vector.tensor_sub(t2[:, H:2 * H], trs[:],
                                             t2[:, 0:H])
                        nc.sync.dma_start(
                            out=tsh[li][g0 + 128 * t:g0 + 128 * (t + 1), :],
                            in_=t2[:])

            qn_state = [0]
            cnt_regs = [nc.gpsimd.alloc_register(f"cntr{i}")
                        for i in range(2 * NQ)]

            def edge_phase(li):
                """agg per group via gather + one-hot matmuls; fp32
                dinv[dst] column scale; GRU epilogue."""
                for g in range(NGRP):
                    gw = GW[g]
                    g0 = g * GRP
                    gcalls = [cl for cl in calls if cl[0] == g]
                    pagg = pp.tile([128, GRP], f32, tag="pagg")
                    nc.vector.memset(pagg[:, :gw], 0.0)
                    for (gg, cc, ioff, ntot, windows, ci, kmin) in gcalls:
                        soff0 = windows[0][0]
                        scw = windows[-1][0] + windows[-1][1] - soff0
                        assert scw <= sslcap, scw
                        ssl = sp.tile([128, sslcap], f16, tag="ssl",
                                      bufs=GBUFS,
                                      name=f"ssl_{li}_{gg}_{cc}_{ioff}")
                        nc.sync.dma_start(out=ssl[:, :scw],
                                          in_=s_d[:, soff0:soff0 + scw])
                        nw = ntot // 128
                        assert nw <= nwmax
                        it = sp.tile([128, nwmax * 8], dt.int16, tag="it",
                                     bufs=GBUFS)
                        nc.sync.dma_start(
                            out=it[:, :ntot // 16],
                            in_=idx_d[:, ioff // 16:(ioff + ntot) // 16])
                        mt = gatp.tile([128, nwmax * 2 * H], f16, tag="mt")
                        # zero chunks that trailing-(-1)-trimmed gathers may
                        # skip on some core: their 0-weight matmul columns
                        # must multiply finite data.
                        if kmin < nw:
                            nc.vector.memset(
                                mt[:, kmin * 2 * H:nw * 2 * H], 0.0)
                        cval = cnt_regs[ci % len(cnt_regs)]
                        nc.gpsimd.reg_load(cval, cnt_t[0:1, ci:ci + 1])
                        nc.gpsimd.dma_gather(
                            out_ap=mt[:, :nw * 2 * H].rearrange(
                                "p (c d) -> p c d", d=2 * H),
                            in_ap=Ttab[li][cc * WIN:(cc + 1) * WIN, :],
                            idxs_ap=it[:, :ntot // 16],
                            num_idxs=ntot, num_idxs_reg=cval,
                            elem_size=2 * H,
                            single_packet=bool(ntot <= 1024),
                            queue_num=qn_state[0])
                        qn_state[0] = (qn_state[0] + 1) % NQ
                        for k, (soff, W, o) in enumerate(windows):
                            nc.tensor.matmul(
                                pagg[:, o:o + W],
                                mt[:, k * 2 * H:k * 2 * H + H],
                                ssl[:, soff - soff0:soff - soff0 + W],
                                start=False, stop=False,
                                skip_group_check=True)
                            nc.tensor.matmul(
                                pagg[:, o:o + W],
                                mt[:, k * 2 * H + H:(k + 1) * 2 * H],
                                ssl[:, soff - soff0:soff - soff0 + W],
                                start=False, stop=False,
                                skip_group_check=True)
                    # fp32 column scale by dinv[dst], then ReLU(+bias)
                    dvb = sp.tile([128, GRP], f32, tag="dvb", bufs=2)
                    nc.sync.dma_start(out=dvb[:, :gw],
                                      in_=dv_rep[:, g0:g0 + gw])
                    hgs = sp.tile([128, GRP], f32, tag="hgs")
                    nc.vector.tensor_mul(hgs[:, :gw], pagg[:, :gw],
                                         dvb[:, :gw])
                    hg = sp.tile([128, GRP], f32, tag="hg")
                    nc.scalar.activation(hg[:, :gw], hgs[:, :gw], ACT.Relu,
                                         bias=bc[li][:], scale=1.0)
                    if _dbg and li == 0:
                        nc.sync.dma_start(out=dbg_hgs[:, g0:g0 + gw],
                                          in_=hgs[:, :gw])
                        nc.sync.dma_start(out=dbg_hg[:, g0:g0 + gw],
                                          in_=hg[:, :gw])
                    # GRU gates
                    pgr = pp2.tile([128, GRP], f32, tag="pg")
                    nc.tensor.matmul(pgr[:, :gw], wih[li][:, 0:H],
                                     hg[:, :gw], start=True, stop=True)
                    rt = sp.tile([128, GRP], f32, tag="rt", bufs=2)
                    nc.scalar.activation(rt[:, :gw], pgr[:, :gw], ACT.Sigmoid,
                                         bias=br[li][:], scale=1.0)
                    pgz = pp2.tile([128, GRP], f32, tag="pg")
                    nc.tensor.matmul(pgz[:, :gw], wih[li][:, H:2 * H],
                                     hg[:, :gw], start=True, stop=True)
                    zt = sp.tile([128, GRP], f32, tag="zt", bufs=2)
                    nc.scalar.activation(zt[:, :gw], pgz[:, :gw], ACT.Sigmoid,
                                         bias=bz[li][:], scale=-1.0)
                    pgn = pp2.tile([128, GRP], f32, tag="pg")
                    nc.tensor.matmul(pgn[:, :gw], wih[li][:, 2 * H:3 * H],
                                     hg[:, :gw], start=True, stop=True)
                    tmp = sp.tile([128, GRP], f32, tag="tmp", bufs=2)
                    nc.vector.tensor_scalar_mul(tmp[:, :gw], rt[:, :gw],
                                                bhn[li][:])
                    st = sp.tile([128, GRP], f32, tag="st", bufs=2)
                    nc.vector.tensor_add(st[:, :gw], pgn[:, :gw], tmp[:, :gw])
                    nt = sp.tile([128, GRP], f32, tag="nt", bufs=2)
                    nc.scalar.activation(nt[:, :gw], st[:, :gw], ACT.Tanh,
                                         bias=bn[li][:], scale=1.0)
                    if li == 0:
                        nc.vector.tensor_mul(hT[:, g0:g0 + gw], zt[:, :gw],
                                             nt[:, :gw])
                    else:
                        h2 = sp.tile([128, GRP], f32, tag="h2", bufs=2)
                        nc.vector.tensor_mul(h2[:, :gw], zt[:, :gw],
                                             nt[:, :gw])
                        for t in range(gw // 128):
                            py = pp3.tile([128, 128], f32, tag="ptr")
                            nc.tensor.matmul(py[:, :2],
                                             h2[:, 128 * t:128 * (t + 1)],
                                             wlin[:], start=True, stop=True)
                            col = g * (GRP // 128) + t
                            nc.vector.tensor_add(
                                y_sb[:, 2 * col:2 * col + 2],
                                py[:, :2], blt[:])

            # ---- layer 0 ----
            dense_to_table(0)
            nc.gpsimd.collective_compute(
                "AllGather", mybir.AluOpType.bypass,
                replica_groups=[list(range(NC))],
                ins=[tsh[0][:]], outs=[Ttab[0][:]])
            edge_phase(0)
            if _dbg:
                nc.sync.dma_start(out=dbg_hT[:], in_=hT[:])
            # ---- layer 1 ----
            dense_to_table(1)
            nc.gpsimd.collective_compute(
                "AllGather", mybir.AluOpType.bypass,
                replica_groups=[list(range(NC))],
                ins=[tsh[1][:]], outs=[Ttab[1][:]])
            edge_phase(1)
            nc.sync.dma_start(out=out[:], in_=y_sb[:])

    _split_excess_waits(nc, bass)
    nc.finalize()
    return nc


def _stage(inputs):
    """Host preprocessing: uniform SPMD call structure + per-core arrays."""
    x = np.asarray(inputs["x"], np.float32)
    edge_index = np.asarray(inputs["edge_index"])
    dinv, per_core = _preprocess(edge_index)

    # SPMD requires one program: pad every core's call structure to the
    # max over cores (same call set, same static ntot, unified window
    # column bounds). True per-core counts are preserved via trailing -1
    # indices (trimmed by the gather ucode) and zero selection columns.
    key_set = sorted({(cl[0], cl[1]) for pc in per_core for cl in pc["calls"]})
    call_shape = {}
    for kk in key_set:
        mx_nt = 0
        for pc in per_core:
            for cl in pc["calls"]:
                if (cl[0], cl[1]) == kk:
                    mx_nt = max(mx_nt, cl[3])
        call_shape[kk] = mx_nt
    # uniform window column bounds across cores: o_u = min o, end = max o+W
    win_b = {}
    for kk in key_set:
        nww = call_shape[kk] // 128
        for wi in range(nww):
            lo, hi = None, None
            for pc in per_core:
                for cl in pc["calls"]:
                    if (cl[0], cl[1]) == kk and wi < len(cl[4]):
                        _, W_, o_ = cl[4][wi]
                        lo = o_ if lo is None else min(lo, o_)
                        hi = o_ + W_ if hi is None else max(hi, o_ + W_)
            if lo is None:
                lo, hi = 0, 1
            win_b[(kk, wi)] = (lo, hi - lo)

    # rebuild uniform per-core streams
    uni = None
    idx_arrs, s_arrs, cnt_arrs = [], [], []
    for pc in per_core:
        bycall = {(cl[0], cl[1]): cl for cl in pc["calls"]}
        idx_cols, s_cols, calls, cnts = [], [], [], []
        ioff = 0
        soff = 0
        for kk in key_set:
            gg, cc = kk
            ntot = call_shape[kk]
            nww = ntot // 128
            windows = []
            if kk in bycall:
                _, _, io0, nt0, ws0 = bycall[kk]
                iflat = pc["idx"][:16].T.reshape(-1)[io0:io0 + nt0]
            else:
                nt0, ws0 = 0, []
                iflat = np.zeros(0, np.int16)
            iful = np.concatenate(
                [iflat, np.full(ntot - nt0, -1, np.int16)])
            true_n = int((iful >= 0).sum())
            idx_cols.append(iful)
            for wi in range(nww):
                o_u, Wp = win_b[(kk, wi)]
                sb = np.zeros((128, Wp), np.float16)
                if wi < len(ws0):
                    s0, W0_, o_ = ws0[wi]
                    sb[:, o_ - o_u:o_ - o_u + W0_] = pc["s"][:, s0:s0 + W0_]
                windows.append((soff, Wp, o_u))
                s_cols.append(sb)
                soff += Wp
            for sc0 in range(0, nww, NWCAP):
                wsub = windows[sc0:sc0 + NWCAP]
                nsub = 128 * len(wsub)
                calls.append((gg, cc, ioff, nsub, wsub, len(calls)))
                cnts.append(max(0, min(true_n - sc0 * 128, nsub)))
                ioff += nsub
        idx_flat = np.concatenate(idx_cols)
        idx_arrs.append(np.tile(idx_flat.reshape(-1, 16).T, (NC, 1)).copy())
        s_arrs.append(np.ascontiguousarray(np.concatenate(s_cols, axis=1)))
        cnt_arrs.append(np.asarray(cnts, np.int32).reshape(1, -1))
        if uni is None:
            nwmax = max(cl[3] // 128 for cl in calls)
            sslcap = 0
            for cl in calls:
                ws = cl[4]
                sslcap = max(sslcap,
                             ws[-1][0] + ws[-1][1] - ws[0][0])
            sslcap = -(-sslcap // 64) * 64
            uni = dict(calls=calls, n_idx=ioff, n_scols=soff, nwmax=nwmax,
                       sslcap=sslcap)

    # per call: min #gather-written chunks over cores (the rest must be
    # zeroed device-side before the gather)
    all_cnts = np.stack([a[0] for a in cnt_arrs])  # [NC, ncalls]
    kmin = np.min(all_cnts // 128, axis=0)
    uni["calls"] = [
        (gg, cc, ioff_, ntot_, ws_, ci_, int(kmin[ci_]))
        for (gg, cc, ioff_, ntot_, ws_, ci_) in uni["calls"]]

    # stage per-core inputs
    W0 = np.asarray(inputs["W0"], np.float32)
    W1 = np.asarray(inputs["W1"], np.float32)
    Wlin = np.asarray(inputs["Wlin"], np.float32)
    in_maps = []
    for c in range(NC):
        ids = np.arange(c * SH, (c + 1) * SH)
        xs = np.zeros((SHP, IN), np.float32)
        xs[:SH] = x[ids]
        xT = np.ascontiguousarray(xs.T)
        bias_stage = {}
        for li in range(2):
            bih = np.asarray(inputs[f"bih{li}"], np.float32)
            bhh = np.asarray(inputs[f"bhh{li}"], np.float32)
            bias_stage[f"bcol{li}"] = np.asarray(
                inputs[f"b{li}"], np.float32).reshape(128, 1)
            bias_stage[f"brc{li}"] = (bih[:H] + bhh[:H]).reshape(128, 1)
            bias_stage[f"bzc{li}"] = (
                -(bih[H:2 * H] + bhh[H:2 * H])).reshape(128, 1)
            bias_stage[f"bnc{li}"] = bih[2 * H:].reshape(128, 1)
            bias_stage[f"bhnc{li}"] = bhh[2 * H:].reshape(128, 1)
        dv = np.zeros(SHP, np.float32)
        dv[:SH] = dinv[ids]
        in_maps.append({
            "xT_hi": xT[:128], "xT_lo": xT[128:],
            "W0T_hi": np.ascontiguousarray(W0.T[:128]),
            "W0T_lo": np.ascontiguousarray(W0.T[128:]),
            "W1T": np.ascontiguousarray(W1.T),
            "WihT0": np.ascontiguousarray(
                np.asarray(inputs["Wih0"], np.float32).T),
            "WihT1": np.ascontiguousarray(
                np.asarray(inputs["Wih1"], np.float32).T),
            "WlinT": np.ascontiguousarray(Wlin.T),
            **bias_stage,
            "dinv_col": np.ascontiguousarray(dv.reshape(SHP // 128, 128).T),
            "dv_rep": np.ascontiguousarray(np.tile(dv[None, :], (128, 1))),
            "blin_t": np.tile(np.asarray(inputs["blin"], np.float32),
                              (128, 1)),
            "idx_d": idx_arrs[c],
            "s_d": s_arrs[c],
            "cnt_d": cnt_arrs[c],
        })
    return uni, in_maps


def kernel(**inputs):
    _install_ntff_hook()
    import concourse.bass as bass
    import concourse.bacc as bacc
    import concourse.mybir as mybir
    import concourse.tile as tile
    from concourse.bass_utils import run_bass_kernel_spmd

    uni, in_maps = _stage(inputs)
    nc = _build_program(bass, bacc, mybir, tile, uni)

    res = run_bass_kernel_spmd(nc, in_maps, list(range(NC)),
                               trace=bool(int(__import__("os").environ.get(
                                   "KERNEL_TRACE", "0"))))
    kernel.last_results = res
    y = np.zeros((N, 2), np.float32)
    for c in range(NC):
        o = res.results[c]["out"]  # [128, 98*2]
        yy = o.reshape(128, SHP // 128, 2).transpose(1, 0, 2).reshape(SHP, 2)
        y[c * SH:(c + 1) * SH] = yy[:SH]
    return y


# revision 26
# speedup vs baseline: 2.5169x; 1.1098x over previous
"""EvolveGCN (2x GCNConv+GRU + linear head) on 8 Trainium2 NeuronCores.

Strategy: dst-sharded graph parallel. Each core owns 12500 destination
nodes (padded to 12544). Dense per-node compute is feature-major
([128 feat partitions, nodes free]) in fp32. The GCN propagate uses a
table of per-node rows dinv[src]*t[src] stored as compensated fp16
hi+lo pairs ([NTAB, 2H] fp16, hi|lo packed per row, 512B/row). Edge
messages are fetched with dma_gather (4-way parallel across the 4 SWDGE
queues / Q7 core pairs), multiplied against host-built one-hot fp16
selection matrices on the PE (2 matmuls per 128-edge window: hi + lo),
accumulated in fp32 PSUM, then column-scaled by dinv[dst] in fp32.
Gather index streams are padded with -1 (the Q7 ucode trims trailing
negatives, so emission cost tracks the true per-core edge count).
Tables are exchanged between layers with one AllGather per layer.
"""
import sys
import types

import numpy as np

sys.path.insert(0, "/opt/trn_rl_repo")

N = 100000
E = 1600000
IN = 165
H = 128
NC = 8
SH = 12500
SHP = 12544           # 98 * 128
NTAB = NC * SHP       # 100352
GRP = 512
NGRP = 25             # 24x512 + 1x256
GW = [512] * 24 + [256]
WIN = 25088           # int16 gather window (4 windows cover NTAB)
NWIN = 4
NWCAP = 21            # max 128-edge windows per gather call
NQ = int(__import__("os").environ.get("KERNEL_NQ", "4"))
GBUFS = 5             # gather tile pool depth (4 queues in flight + 1)


def _install_ntff_hook():
    if "antenv.axon_hooks" in sys.modules:
        return
    try:
        import antenv
        mod = types.ModuleType("antenv.axon_hooks")
        mod._hook = None
        mod.set_axon_ntff_profile_hook = lambda h: setattr(mod, "_hook", h)
        mod.get_axon_ntff_profile_hook = lambda: mod._hook
        sys.modules["antenv.axon_hooks"] = mod
        antenv.axon_hooks = mod
        from trn_agent_boot.trn_boot import _ntff_profile_via_ctypes
        mod.set_axon_ntff_profile_hook(
            _ntff_profile_via_ctypes("/opt/axon/libaxon_pjrt.so"))
    except Exception:
        pass


def _split_excess_waits(nc, bass, max_waits=1, kinds=("InstDrain",)):
    """This walrus build rejects InstDrain with >1 sem waits; hoist extras
    onto standalone event-semaphore instructions placed just before."""
    wait_op_map = {"sem-ge-imm": "sem-ge", "sem-eq-imm": "sem-eq"}
    for bb in nc.main_func.blocks:
        insts = bb.instructions
        i = 0
        while i < len(insts):
            ins = insts[i]
            if (type(ins).__name__ in kinds and ins.sync_info is not None
                    and len(ins.sync_info.on_wait) > max_waits):
                waits = list(ins.sync_info.on_wait)
                ins.sync_info.on_wait = waits[:max_waits]
                eng = nc.engines[ins.engine]
                new_insts = []
                for w in waits[max_waits:]:
                    sem = bass.SemaphoreHandle(w.ant_name or "s", w.id)
                    bi = eng.wait_op(sem, w.wait_value, wait_op_map[w.wait_mode])
                    popped = None
                    for b2 in nc.main_func.blocks:
                        if b2.instructions and b2.instructions[-1] is bi.ins:
                            popped = b2.instructions.pop()
                            break
                    assert popped is bi.ins
                    new_insts.append(popped)
                for k, ni in enumerate(new_insts):
                    insts.insert(i + k, ni)
                i += len(new_insts)
            i += 1


def _newid(v):
    return (v // SH) * SHP + (v % SH)


def _preprocess(edge_index):
    """Build per-core gather index streams + one-hot selection streams."""
    e0 = np.asarray(edge_index[0], dtype=np.int64)
    e1 = np.asarray(edge_index[1], dtype=np.int64)
    deg = np.bincount(e1, minlength=N).astype(np.float64) + 1.0
    dinv = (1.0 / np.sqrt(deg)).astype(np.float32)

    src = np.concatenate([e0, np.arange(N, dtype=np.int64)])
    dst = np.concatenate([e1, np.arange(N, dtype=np.int64)])

    src_n = _newid(src)
    core = dst // SH
    dstloc = dst % SH

    per_core = []
    for c in range(NC):
        m = core == c
        s, dl = src_n[m], dstloc[m]
        g = dl // GRP
        ch = s // WIN
        order = np.lexsort((s, dl, ch, g))
        s, dl, g, ch = s[order], dl[order], g[order], ch[order]

        idx_cols, s_cols, calls = [], [], []
        key = g * NWIN + ch
        bounds = np.flatnonzero(np.diff(key)) + 1
        starts = np.concatenate([[0], bounds])
        ends = np.concatenate([bounds, [len(s)]])
        idx_off = 0
        s_off = 0
        for a, b in zip(starts, ends):
            gg, cc = int(g[a]), int(ch[a])
            n = b - a
            npad = -n % 128
            ntot = n + npad
            ss = np.concatenate(
                [s[a:b] - cc * WIN, np.full(npad, -1, np.int64)])
            dd = np.concatenate([dl[a:b], np.full(npad, dl[b - 1], np.int64)])
            vv = np.concatenate(
                [np.ones(n, np.float32), np.zeros(npad, np.float32)])
            idx_cols.append(ss.astype(np.int16))
            windows = []
            for k in range(ntot // 128):
                dk = dd[k * 128:(k + 1) * 128]
                vk = vv[k * 128:(k + 1) * 128]
                lo = int(dk.min())
                W = int(dk.max()) - lo + 1
                sb = np.zeros((128, W), np.float16)
                sb[np.arange(128), dk - lo] = vk
                s_cols.append(sb)
                windows.append((s_off, W, lo - gg * GRP))
                s_off += W
            calls.append((gg, cc, idx_off, ntot, windows))
            idx_off += ntot

        idx_flat = np.concatenate(idx_cols)
        idx_wrapped = np.tile(idx_flat.reshape(-1, 16).T, (NC, 1)).copy()
        s_flat = np.ascontiguousarray(np.concatenate(s_cols, axis=1))
        per_core.append(dict(idx=idx_wrapped, s=s_flat, calls=calls,
                             n_idx=idx_off, n_scols=s_off))
    return dinv, per_core


def _build_program(bass, bacc, mybir, tile, meta):
    """Build the SPMD bass program (identical structure for all cores)."""
    nc = bacc.Bacc("TRN2", target_bir_lowering=False, debug=False,
                   num_swdge_queues=NQ)
    dt = mybir.dt
    f32 = dt.float32
    f16 = dt.float16

    def din(name, shape, dtype=f32):
        return nc.dram_tensor(name, shape, dtype, kind="ExternalInput").ap()

    n_idx, n_scols, calls = meta["n_idx"], meta["n_scols"], meta["calls"]
    nwmax = meta["nwmax"]
    sslcap = meta["sslcap"]

    xT_hi = din("xT_hi", [128, SHP])
    xT_lo = din("xT_lo", [IN - 128, SHP])
    W0T_hi = din("W0T_hi", [128, H])
    W0T_lo = din("W0T_lo", [IN - 128, H])
    W1T = din("W1T", [H, H])
    WihT = [din(f"WihT{li}", [H, 3 * H]) for li in range(2)]
    WlinT = din("WlinT", [H, 2])
    bcol = [din(f"bcol{li}", [128, 1]) for li in range(2)]
    brc = [din(f"brc{li}", [128, 1]) for li in range(2)]
    bzc = [din(f"bzc{li}", [128, 1]) for li in range(2)]
    bnc = [din(f"bnc{li}", [128, 1]) for li in range(2)]
    bhnc = [din(f"bhnc{li}", [128, 1]) for li in range(2)]
    dinv_col = din("dinv_col", [128, SHP // 128])
    dv_rep = din("dv_rep", [128, SHP])
    blin_t = din("blin_t", [128, 2])
    idx_d = din("idx_d", [128, n_idx // 16], dt.int16)
    s_d = din("s_d", [128, n_scols], f16)
    ncalls = len(calls)
    cnt_d = din("cnt_d", [1, ncalls], dt.int32)

    out = nc.dram_tensor("out", [128, (SHP // 128) * 2], f32,
                         kind="ExternalOutput").ap()
    _dbg = bool(int(__import__("os").environ.get("KERNEL_DEBUG", "0")))
    if _dbg:
        dbg_hgs = nc.dram_tensor("dbg_hgs", [128, SHP], f32,
                                 kind="ExternalOutput").ap()
        dbg_hg = nc.dram_tensor("dbg_hg", [128, SHP], f32,
                                kind="ExternalOutput").ap()
        dbg_hT = nc.dram_tensor("dbg_hT", [128, SHP], f32,
                                kind="ExternalOutput").ap()

    tsh = [nc.dram_tensor(f"tshard{li}", [SHP, 2 * H], f16) for li in range(2)]
    Ttab = [nc.dram_tensor(f"Ttab{li}", [NTAB, 2 * H], f16,
                           addr_space="Shared") for li in range(2)]

    from concourse.masks import make_identity

    with tile.TileContext(nc) as tc:
        with (
            tc.tile_pool(name="const", bufs=1) as cp,
            tc.tile_pool(name="sb", bufs=3) as sp,
            tc.tile_pool(name="big", bufs=1) as bigp,
            tc.tile_pool(name="gat", bufs=GBUFS) as gatp,
            tc.tile_pool(name="ps", bufs=2, space="PSUM") as pp,
            tc.tile_pool(name="ps2", bufs=2, space="PSUM") as pp2,
            tc.tile_pool(name="ps3", bufs=2, space="PSUM") as pp3,
            tc.tile_pool(name="dram", bufs=1, space="DRAM") as _dp,
        ):
            ident = cp.tile([128, 128], f32)
            make_identity(nc, ident[:])
            # resident constants
            w0hi = cp.tile([128, H], f32)
            nc.sync.dma_start(out=w0hi[:], in_=W0T_hi[:])
            w0lo = cp.tile([IN - 128, H], f32)
            nc.sync.dma_start(out=w0lo[:], in_=W0T_lo[:])
            w1 = cp.tile([H, H], f32)
            nc.sync.dma_start(out=w1[:], in_=W1T[:])
            wih = [cp.tile([H, 3 * H], f32, tag=f"wih{li}", name=f"wih{li}")
                   for li in range(2)]
            for li in range(2):
                nc.sync.dma_start(out=wih[li][:], in_=WihT[li][:])
            wlin = cp.tile([H, 2], f32)
            nc.sync.dma_start(out=wlin[:], in_=WlinT[:])
            bc = [cp.tile([128, 1], f32, name=f"bc{li}") for li in range(2)]
            br = [cp.tile([128, 1], f32, name=f"br{li}") for li in range(2)]
            bz = [cp.tile([128, 1], f32, name=f"bz{li}") for li in range(2)]
            bn = [cp.tile([128, 1], f32, name=f"bn{li}") for li in range(2)]
            bhn = [cp.tile([128, 1], f32, name=f"bhn{li}") for li in range(2)]
            for li in range(2):
                nc.sync.dma_start(out=bc[li][:], in_=bcol[li][:])
                nc.sync.dma_start(out=br[li][:], in_=brc[li][:])
                nc.sync.dma_start(out=bz[li][:], in_=bzc[li][:])
                nc.sync.dma_start(out=bn[li][:], in_=bnc[li][:])
                nc.sync.dma_start(out=bhn[li][:], in_=bhnc[li][:])
            dvc = cp.tile([128, SHP // 128], f32)
            nc.sync.dma_start(out=dvc[:], in_=dinv_col[:])
            blt = cp.tile([128, 2], f32)
            nc.sync.dma_start(out=blt[:], in_=blin_t[:])
            cnt_t = cp.tile([1, ncalls], dt.int32)
            nc.sync.dma_start(out=cnt_t[:], in_=cnt_d[:])

            hT = bigp.tile([128, SHP], f32, tag="hT")      # h1.T resident
            y_sb = bigp.tile([128, (SHP // 128) * 2], f32, tag="ysb")

            ACT = mybir.ActivationFunctionType



            def dense_to_table(li):
                """t.T = W @ h.T per group; transpose; scale dinv;
                split fp16 hi/lo; DMA shard."""
                for g in range(NGRP):
                    gw = GW[g]
                    g0 = g * GRP
                    pt = pp.tile([128, GRP], f32, tag="pt")
                    if li == 0:
                        xh = sp.tile([128, GRP], f32, tag="xh")
                        nc.sync.dma_start(out=xh[:, :gw],
                                          in_=xT_hi[:, g0:g0 + gw])
                        xl = sp.tile([IN - 128, GRP], f32, tag="xl")
                        nc.sync.dma_start(out=xl[:, :gw],
                                          in_=xT_lo[:, g0:g0 + gw])
                        nc.tensor.matmul(pt[:, :gw], w0hi[:], xh[:, :gw],
                                         start=True, stop=False)
                        nc.tensor.matmul(pt[:, :gw], w0lo[:], xl[:, :gw],
                                         start=False, stop=True)
                    else:
                        nc.tensor.matmul(pt[:, :gw], w1[:],
                                         hT[:, g0:g0 + gw],
                                         start=True, stop=True)
                    tsb = sp.tile([128, GRP], f32, tag="tsb")
                    nc.scalar.copy(tsb[:, :gw], pt[:, :gw])
                    for t in range(gw // 128):
                        ptr = pp3.tile([128, 128], f32, tag="ptr")
                        nc.tensor.transpose(
                            out=ptr[:], in_=tsb[:, 128 * t:128 * (t + 1)],
                            identity=ident[:])
                        col = g * (GRP // 128) + t
                        t2 = sp.tile([128, 2 * H], f16, tag="t2")
                        nc.scalar.mul(t2[:, 0:H], ptr[:],
                                      dvc[:, col:col + 1])
                        nc.vector.scalar_tensor_tensor(
                            t2[:, H:2 * H], ptr[:], dvc[:, col:col + 1],
                            t2[:, 0:H], mybir.AluOpType.mult,
                            mybir.AluOpType.subtract)
                        nc.sync.dma_start(
                            out=tsh[li][g0 + 128 * t:g0 + 128 * (t + 1), :],
                            in_=t2[:])

            qn_state = [0]
            cnt_regs = [nc.gpsimd.alloc_register(f"cntr{i}")
                        for i in range(2 * NQ)]

            def edge_phase(li):
                """agg per group via gather + one-hot matmuls; fp32
                dinv[dst] column scale; GRU epilogue."""
                for g in range(NGRP):
                    gw = GW[g]
                    g0 = g * GRP
                    gcalls = [cl for cl in calls if cl[0] == g]
                    pagg = pp.tile([128, GRP], f32, tag="pagg")
                    nc.vector.memset(pagg[:, :gw], 0.0)
                    for (gg, cc, ioff, ntot, windows, ci, kmin) in gcalls:
                        soff0 = windows[0][0]
                        scw = windows[-1][0] + windows[-1][1] - soff0
                        assert scw <= sslcap, scw
                        ssl = sp.tile([128, sslcap], f16, tag="ssl",
                                      bufs=GBUFS,
                                      name=f"ssl_{li}_{gg}_{cc}_{ioff}")
                        nc.sync.dma_start(out=ssl[:, :scw],
                                          in_=s_d[:, soff0:soff0 + scw])
                        nw = ntot // 128
                        assert nw <= nwmax
                        it = sp.tile([128, nwmax * 8], dt.int16, tag="it",
                                     bufs=GBUFS)
                        nc.sync.dma_start(
                            out=it[:, :ntot // 16],
                            in_=idx_d[:, ioff // 16:(ioff + ntot) // 16])
                        mt = gatp.tile([128, nwmax * 2 * H], f16, tag="mt")
                        # zero chunks that trailing-(-1)-trimmed gathers may
                        # skip on some core: their 0-weight matmul columns
                        # must multiply finite data.
                        if kmin < nw:
                            nc.scalar.memzero(
                                mt[:, kmin * 2 * H:nw * 2 * H])
                        cval = cnt_regs[ci % len(cnt_regs)]
                        nc.gpsimd.reg_load(cval, cnt_t[0:1, ci:ci + 1])
                        nc.gpsimd.dma_gather(
                            out_ap=mt[:, :nw * 2 * H].rearrange(
                                "p (c d) -> p c d", d=2 * H),
                            in_ap=Ttab[li][cc * WIN:(cc + 1) * WIN, :],
                            idxs_ap=it[:, :ntot // 16],
                            num_idxs=ntot, num_idxs_reg=cval,
                            elem_size=2 * H,
                            single_packet=bool(ntot <= 1024),
                            queue_num=qn_state[0])
                        qn_state[0] = (qn_state[0] + 1) % NQ
                        for k, (soff, W, o) in enumerate(windows):
                            nc.tensor.matmul(
                                pagg[:, o:o + W],
                                mt[:, k * 2 * H:k * 2 * H + H],
                                ssl[:, soff - soff0:soff - soff0 + W],
                                start=False, stop=False,
                                skip_group_check=True)
                            nc.tensor.matmul(
                                pagg[:, o:o + W],
                                mt[:, k * 2 * H + H:(k + 1) * 2 * H],
                                ssl[:, soff - soff0:soff - soff0 + W],
                                start=False, stop=False,
                                skip_group_check=True)
                    # fp32 column scale by dinv[dst], then ReLU(+bias)
                    dvb = sp.tile([128, GRP], f32, tag="dvb", bufs=2)
                    nc.sync.dma_start(out=dvb[:, :gw],
                                      in_=dv_rep[:, g0:g0 + gw])
                    hgs = sp.tile([128, GRP], f32, tag="hgs")
                    nc.vector.tensor_mul(hgs[:, :gw], pagg[:, :gw],
                                         dvb[:, :gw])
                    hg = sp.tile([128, GRP], f32, tag="hg")
                    nc.scalar.activation(hg[:, :gw], hgs[:, :gw], ACT.Relu,
                                         bias=bc[li][:], scale=1.0)
                    if _dbg and li == 0:
                        nc.sync.dma_start(out=dbg_hgs[:, g0:g0 + gw],
                                          in_=hgs[:, :gw])
                        nc.sync.dma_start(out=dbg_hg[:, g0:g0 + gw],
                                          in_=hg[:, :gw])
                    # GRU gates
                    pgr = pp2.tile([128, GRP], f32, tag="pg")
                    nc.tensor.matmul(pgr[:, :gw], wih[li][:, 0:H],
                                     hg[:, :gw], start=True, stop=True)
                    rt = sp.tile([128, GRP], f32, tag="rt", bufs=2)
                    nc.scalar.activation(rt[:, :gw], pgr[:, :gw], ACT.Sigmoid,
                                         bias=br[li][:], scale=1.0)
                    pgz = pp2.tile([128, GRP], f32, tag="pg")
                    nc.tensor.matmul(pgz[:, :gw], wih[li][:, H:2 * H],
                                     hg[:, :gw], start=True, stop=True)
                    zt = sp.tile([128, GRP], f32, tag="zt", bufs=2)
                    nc.scalar.activation(zt[:, :gw], pgz[:, :gw], ACT.Sigmoid,
                                         bias=bz[li][:], scale=-1.0)
                    pgn = pp2.tile([128, GRP], f32, tag="pg")
                    nc.tensor.matmul(pgn[:, :gw], wih[li][:, 2 * H:3 * H],
                                     hg[:, :gw], start=True, stop=True)
                    st = sp.tile([128, GRP], f32, tag="st", bufs=2)
                    nc.vector.scalar_tensor_tensor(
                        st[:, :gw], rt[:, :gw], bhn[li][:], pgn[:, :gw],
                        mybir.AluOpType.mult, mybir.AluOpType.add)
                    nt = sp.tile([128, GRP], f32, tag="nt", bufs=2)
                    nc.scalar.activation(nt[:, :gw], st[:, :gw], ACT.Tanh,
                                         bias=bn[li][:], scale=1.0)
                    if li == 0:
                        nc.vector.tensor_mul(hT[:, g0:g0 + gw], zt[:, :gw],
                                             nt[:, :gw])
                    else:
                        h2 = sp.tile([128, GRP], f32, tag="h2", bufs=2)
                        nc.vector.tensor_mul(h2[:, :gw], zt[:, :gw],
                                             nt[:, :gw])
                        for t in range(gw // 128):
                            py = pp3.tile([128, 128], f32, tag="ptr")
                            nc.tensor.matmul(py[:, :2],
                                             h2[:, 128 * t:128 * (t + 1)],
                                             wlin[:], start=True, stop=True)
                            col = g * (GRP // 128) + t
                            nc.vector.tensor_add(
                                y_sb[:, 2 * col:2 * col + 2],
                                py[:, :2], blt[:])

            # ---- layer 0 ----
            dense_to_table(0)
            nc.gpsimd.collective_compute(
                "AllGather", mybir.AluOpType.bypass,
                replica_groups=[list(range(NC))],
                ins=[tsh[0][:]], outs=[Ttab[0][:]])
            edge_phase(0)
            if _dbg:
                nc.sync.dma_start(out=dbg_hT[:], in_=hT[:])
            # ---- layer 1 ----
            dense_to_table(1)
            nc.gpsimd.collective_compute(
                "AllGather", mybir.AluOpType.bypass,
                replica_groups=[list(range(NC))],
                ins=[tsh[1][:]], outs=[Ttab[1][:]])
            edge_phase(1)
            nc.sync.dma_start(out=out[:], in_=y_sb[:])

    _split_excess_waits(nc, bass)
    nc.finalize()
    return nc


def _stage(inputs):
    """Host preprocessing: uniform SPMD call structure + per-core arrays."""
    x = np.asarray(inputs["x"], np.float32)
    edge_index = np.asarray(inputs["edge_index"])
    dinv, per_core = _preprocess(edge_index)

    # SPMD requires one program: pad every core's call structure to the
    # max over cores (same call set, same static ntot, unified window
    # column bounds). True per-core counts are preserved via trailing -1
    # indices (trimmed by the gather ucode) and zero selection columns.
    key_set = sorted({(cl[0], cl[1]) for pc in per_core for cl in pc["calls"]})
    call_shape = {}
    for kk in key_set:
        mx_nt = 0
        for pc in per_core:
            for cl in pc["calls"]:
                if (cl[0], cl[1]) == kk:
                    mx_nt = max(mx_nt, cl[3])
        call_shape[kk] = mx_nt
    # uniform window column bounds across cores: o_u = min o, end = max o+W
    win_b = {}
    for kk in key_set:
        nww = call_shape[kk] // 128
        for wi in range(nww):
            lo, hi = None, None
            for pc in per_core:
                for cl in pc["calls"]:
                    if (cl[0], cl[1]) == kk and wi < len(cl[4]):
                        _, W_, o_ = cl[4][wi]
                        lo = o_ if lo is None else min(lo, o_)
                        hi = o_ + W_ if hi is None else max(hi, o_ + W_)
            if lo is None:
                lo, hi = 0, 1
            win_b[(kk, wi)] = (lo, hi - lo)

    # rebuild uniform per-core streams
    uni = None
    idx_arrs, s_arrs, cnt_arrs = [], [], []
    for pc in per_core:
        bycall = {(cl[0], cl[1]): cl for cl in pc["calls"]}
        idx_cols, s_cols, calls, cnts = [], [], [], []
        ioff = 0
        soff = 0
        for kk in key_set:
            gg, cc = kk
            ntot = call_shape[kk]
            nww = ntot // 128
            windows = []
            if kk in bycall:
                _, _, io0, nt0, ws0 = bycall[kk]
                iflat = pc["idx"][:16].T.reshape(-1)[io0:io0 + nt0]
            else:
                nt0, ws0 = 0, []
                iflat = np.zeros(0, np.int16)
            iful = np.concatenate(
                [iflat, np.full(ntot - nt0, -1, np.int16)])
            true_n = int((iful >= 0).sum())
            idx_cols.append(iful)
            for wi in range(nww):
                o_u, Wp = win_b[(kk, wi)]
                sb = np.zeros((128, Wp), np.float16)
                if wi < len(ws0):
                    s0, W0_, o_ = ws0[wi]
                    sb[:, o_ - o_u:o_ - o_u + W0_] = pc["s"][:, s0:s0 + W0_]
                windows.append((soff, Wp, o_u))
                s_cols.append(sb)
                soff += Wp
            for sc0 in range(0, nww, NWCAP):
                wsub = windows[sc0:sc0 + NWCAP]
                nsub = 128 * len(wsub)
                calls.append((gg, cc, ioff, nsub, wsub, len(calls)))
                cnts.append(max(0, min(true_n - sc0 * 128, nsub)))
                ioff += nsub
        idx_flat = np.concatenate(idx_cols)
        idx_arrs.append(np.tile(idx_flat.reshape(-1, 16).T, (NC, 1)).copy())
        s_arrs.append(np.ascontiguousarray(np.concatenate(s_cols, axis=1)))
        cnt_arrs.append(np.asarray(cnts, np.int32).reshape(1, -1))
        if uni is None:
            nwmax = max(cl[3] // 128 for cl in calls)
            sslcap = 0
            for cl in calls:
                ws = cl[4]
                sslcap = max(sslcap,
                             ws[-1][0] + ws[-1][1] - ws[0][0])
            sslcap = -(-sslcap // 64) * 64
            uni = dict(calls=calls, n_idx=ioff, n_scols=soff, nwmax=nwmax,
                       sslcap=sslcap)

    # per call: min #gather-written chunks over cores (the rest must be
    # zeroed device-side before the gather)
    all_cnts = np.stack([a[0] for a in cnt_arrs])  # [NC, ncalls]
    kmin = np.min(all_cnts // 128, axis=0)
    uni["calls"] = [
        (gg, cc, ioff_, ntot_, ws_, ci_, int(kmin[ci_]))
        for (gg, cc, ioff_, ntot_, ws_, ci_) in uni["calls"]]

    # stage per-core inputs
    W0 = np.asarray(inputs["W0"], np.float32)
    W1 = np.asarray(inputs["W1"], np.float32)
    Wlin = np.asarray(inputs["Wlin"], np.float32)
    in_maps = []
    for c in range(NC):
        ids = np.arange(c * SH, (c + 1) * SH)
        xs = np.zeros((SHP, IN), np.float32)
        xs[:SH] = x[ids]
        xT = np.ascontiguousarray(xs.T)
        bias_stage = {}
        for li in range(2):
            bih = np.asarray(inputs[f"bih{li}"], np.float32)
            bhh = np.asarray(inputs[f"bhh{li}"], np.float32)
            bias_stage[f"bcol{li}"] = np.asarray(
                inputs[f"b{li}"], np.float32).reshape(128, 1)
            bias_stage[f"brc{li}"] = (bih[:H] + bhh[:H]).reshape(128, 1)
            bias_stage[f"bzc{li}"] = (
                -(bih[H:2 * H] + bhh[H:2 * H])).reshape(128, 1)
            bias_stage[f"bnc{li}"] = bih[2 * H:].reshape(128, 1)
            bias_stage[f"bhnc{li}"] = bhh[2 * H:].reshape(128, 1)
        dv = np.zeros(SHP, np.float32)
        dv[:SH] = dinv[ids]
        in_maps.append({
            "xT_hi": xT[:128], "xT_lo": xT[128:],
            "W0T_hi": np.ascontiguousarray(W0.T[:128]),
            "W0T_lo": np.ascontiguousarray(W0.T[128:]),
            "W1T": np.ascontiguousarray(W1.T),
            "WihT0": np.ascontiguousarray(
                np.asarray(inputs["Wih0"], np.float32).T),
            "WihT1": np.ascontiguousarray(
                np.asarray(inputs["Wih1"], np.float32).T),
            "WlinT": np.ascontiguousarray(Wlin.T),
            **bias_stage,
            "dinv_col": np.ascontiguousarray(dv.reshape(SHP // 128, 128).T),
            "dv_rep": np.ascontiguousarray(np.tile(dv[None, :], (128, 1))),
            "blin_t": np.tile(np.asarray(inputs["blin"], np.float32),
                              (128, 1)),
            "idx_d": idx_arrs[c],
            "s_d": s_arrs[c],
            "cnt_d": cnt_arrs[c],
        })
    return uni, in_maps


def kernel(**inputs):
    _install_ntff_hook()
    import concourse.bass as bass
    import concourse.bacc as bacc
    import concourse.mybir as mybir
    import concourse.tile as tile
    from concourse.bass_utils import run_bass_kernel_spmd

    uni, in_maps = _stage(inputs)
    nc = _build_program(bass, bacc, mybir, tile, uni)

    res = run_bass_kernel_spmd(nc, in_maps, list(range(NC)),
                               trace=bool(int(__import__("os").environ.get(
                                   "KERNEL_TRACE", "0"))))
    kernel.last_results = res
    y = np.zeros((N, 2), np.float32)
    for c in range(NC):
        o = res.results[c]["out"]  # [128, 98*2]
        yy = o.reshape(128, SHP // 128, 2).transpose(1, 0, 2).reshape(SHP, 2)
        y[c * SH:(c + 1) * SH] = yy[:SH]
    return y
